# revision 27
# baseline (speedup 1.0000x reference)
"""Trainium2 Bass kernel for nn_AlternatingSimple (GNN message passing).

Self-contained: takes full (unsharded) numpy inputs, shards across 8
NeuronCores (data-parallel over graphs), runs one SPMD Bass/Tile NEFF,
gathers the full output.
"""
import sys, os
sys.path.insert(0, '/opt/trn_rl_repo')
import numpy as np
import ml_dtypes

import concourse.bass as bass
import concourse.bacc as bacc
import concourse.tile as tile
import concourse.mybir as mybir
from concourse import library_config
from concourse.bass_utils import run_bass_kernel_spmd

BF16 = mybir.dt.bfloat16
FP16 = mybir.dt.float16
F32 = mybir.dt.float32
I16 = mybir.dt.int16
NPBF16 = ml_dtypes.bfloat16

NCORES = 8
B = 500
GPC = 64          # graphs per core (padded: 8*64 = 512 >= 500)
GTOT = NCORES * GPC
F_X = 64
F_U = 32
F_OUT = 2
N_STEPS = 2
WIN = 256          # aggregation node window (2 chunks of 128)

ACT = mybir.ActivationFunctionType
ALU = mybir.AluOpType


def _wrap_idx(a):
    """int16 index array -> [128, ceil(n/16)] wrapped in 16 partitions, replicated 8x."""
    n = len(a)
    ncol = (n + 15) // 16
    w = np.zeros((16, ncol), np.int16)
    w[np.arange(n) % 16, np.arange(n) // 16] = a
    return np.tile(w, (8, 1))


def _bf(a):
    return np.asarray(a, np.float32).astype(NPBF16)


def _group_edges(dst_local, n_chunks, G):
    """Assign dst-sorted edges to n_chunks groups of capacity G*128; group g may
    only hold edges with dst_local in [128g, 128g+WIN). Returns groups or None."""
    cap = G * 128
    groups = [[] for _ in range(n_chunks)]
    chunk_of = dst_local // 128
    order = np.arange(len(dst_local))
    for c in range(n_chunks):
        idxs = order[chunk_of == c]
        pos = 0
        if c >= 1:
            spare = cap - len(groups[c - 1])
            take = min(spare, len(idxs))
            groups[c - 1].extend(idxs[:take])
            pos = take
        rest = idxs[pos:]
        if len(groups[c]) + len(rest) > cap:
            return None
        groups[c].extend(rest)
    return groups


def _prep(inputs):
    """Host-side sharding and constant packing. Returns (meta, in_maps)."""
    gi = lambda k: np.asarray(inputs[k])
    x = [gi('x1').astype(np.float32), gi('x2').astype(np.float32)]
    ei = [gi('edge_index1').astype(np.int64), gi('edge_index2').astype(np.int64)]
    ee = [gi('e1').astype(np.float32), gi('e2').astype(np.float32)]
    uu = [gi('u1').astype(np.float32), gi('u2').astype(np.float32)]
    bt = [gi('batch1').astype(np.int64), gi('batch2').astype(np.int64)]

    params = {}
    for nm in ['params_edge', 'params_node', 'params_attn', 'params_glob', 'params_out']:
        Ws = []
        for (Wt, bb) in inputs[nm]:
            Ws.append(np.asarray(Wt, np.float32))
            assert np.allclose(np.asarray(bb), 0.0), f"nonzero bias in {nm}"
        params[nm] = Ws

    node_start = np.zeros((2, NCORES + 1), np.int64)
    for d in range(2):
        node_start[d] = np.searchsorted(bt[d], np.arange(NCORES + 1) * GPC)
    ncnt = node_start[:, 1:] - node_start[:, :-1]
    N_pad = int(np.ceil(ncnt.max() / 128) * 128)
    assert NCORES * N_pad < 32768
    NCH = N_pad // 128

    shards = [[None] * NCORES for _ in range(2)]
    for d in range(2):
        src_g, dst_g = ei[d][0], ei[d][1]
        core_of_dst = bt[d][dst_g] // GPC
        for c in range(NCORES):
            m = np.nonzero(core_of_dst == c)[0]
            dl = (dst_g[m] - node_start[d][c]).astype(np.int64)
            o = np.argsort(dl, kind='stable')
            shards[d][c] = (m[o], dl[o])

    G = max(2, int(np.ceil(max(len(s[0]) for dd in shards for s in dd) / (128 * NCH))))
    while True:
        if all(_group_edges(shards[d][c][1], NCH, G) is not None
               for d in range(2) for c in range(NCORES)):
            break
        G += 1
    NE_CH = NCH * G
    NE_CH_PAD = ((NE_CH + 3) // 4) * 4
    E_eff = NE_CH_PAD * 128
    NT = E_eff // 512

    meta = dict(N_pad=N_pad, NCH=NCH, G=G, NE_CH=NE_CH, NE_CH_PAD=NE_CH_PAD,
                E_eff=E_eff, NT=NT)

    in_maps = [dict() for _ in range(NCORES)]

    We = params['params_edge']; Wn = params['params_node']
    Wa = params['params_attn']; Wg = params['params_glob']; Wo = params['params_out']
    shared = {
        'we1a': _bf(We[0][0:64]), 'we1b': _bf(We[0][64:128]),
        'we1u1p': _bf(np.concatenate([We[0][224:256], np.zeros((32, 128), np.float32)], 0)),
        'we1ue': _bf(np.concatenate([We[0][192:256], We[0][128:192]], 0)),
        'we1e': _bf(We[0][128:192]),
        'w1ua': _bf(We[0][192:224]), 'w1ub': _bf(We[0][224:256]),
        'we2': _bf(We[1]), 'we3': _bf(We[2]),
        'wn1x': _bf(Wn[0][0:64]), 'wn1a': _bf(Wn[0][64:128]),
        'wn1ua': np.asarray(Wn[0][128:160], np.float32),
        'wn1ub': np.asarray(Wn[0][160:192], np.float32),
        'wn2': _bf(Wn[1]), 'wn3': _bf(Wn[2]),
        'wa1': _bf(Wa[0]), 'wa2': _bf(Wa[1]), 'wa3': _bf(Wa[2]),
        'wg1p': np.asarray(Wg[0][0:64], np.float32),
        'wg1a': np.asarray(Wg[0][64:96], np.float32),
        'wg1b': np.asarray(Wg[0][96:128], np.float32),
        'wg2': np.asarray(Wg[1], np.float32), 'wg3': np.asarray(Wg[2], np.float32),
        'wo1a': np.asarray(Wo[0][0:32], np.float32),
        'wo1b': np.asarray(Wo[0][32:64], np.float32),
        'wo2': np.asarray(Wo[1], np.float32), 'wo3': np.asarray(Wo[2], np.float32),
        'iota': np.broadcast_to(np.arange(WIN, dtype=np.float16), (128, WIN)).copy(),
        'ident': np.eye(128, dtype=np.float32),
        'identb': np.eye(128, dtype=np.float32).astype(NPBF16),
        'iotaG': (np.arange(128)[:, None] + 128 * np.arange(4)[None, :]).astype(np.float32),
        'wg1pb': _bf(Wg[0][0:64]), 'wg1ab': _bf(Wg[0][64:96]),
        'wg1bb': _bf(Wg[0][96:128]), 'wg2b': _bf(Wg[1]), 'wg3b': _bf(Wg[2]),
    }

    for d in range(2):
        tblx = np.zeros((NCORES * N_pad, 128), NPBF16)
        for c in range(NCORES):
            n0, n1 = node_start[d][c], node_start[d][c + 1]
            tblx[c * N_pad: c * N_pad + (n1 - n0), 0:64] = _bf(x[d][n0:n1])
        shared[f'xtbl{d}'] = tblx

    u_pad = [np.zeros((GTOT, F_U), np.float32) for _ in range(2)]
    for d in range(2):
        u_pad[d][:B] = uu[d]
    tblu0 = np.zeros((GTOT, 128), NPBF16)
    tblu0[:, 0:32] = _bf(u_pad[0]); tblu0[:, 32:64] = _bf(u_pad[1])
    shared['utbl0'] = tblu0
    shared['uF0'] = _bf(u_pad[0].T)
    shared['uF1'] = _bf(u_pad[1].T)

    def tbl_row(d, nodes):
        cor = bt[d][nodes] // GPC
        return (cor * N_pad + nodes - node_start[d][cor]).astype(np.int16)

    for c in range(NCORES):
        im = in_maps[c]
        im.update(shared)
        for d in range(2):
            perm, dl = shards[d][c]
            groups = _group_edges(dl, NCH, G)
            e_src = np.zeros(E_eff, np.int16)
            e_dstt = np.zeros(E_eff, np.int16)
            e_gsrc = np.zeros(E_eff, np.int16)
            dloc = np.full(E_eff, -1.0, np.float32)
            rcp = np.zeros(E_eff, np.float32)
            eTv = np.zeros((E_eff, 64), NPBF16)
            deg = np.bincount(dl, minlength=N_pad).astype(np.float32)
            rdeg = 1.0 / np.maximum(deg, 1.0)
            for g in range(NCH):
                sel = np.asarray(groups[g], dtype=np.int64)
                o = g * G * 128
                if len(sel):
                    gp = perm[sel]
                    gd = dl[sel]
                    e_src[o:o + len(sel)] = tbl_row(d, ei[d][0][gp])
                    e_dstt[o:o + len(sel)] = (c * N_pad + gd).astype(np.int16)
                    e_gsrc[o:o + len(sel)] = bt[d][ei[d][0][gp]].astype(np.int16)
                    dloc[o:o + len(sel)] = gd - 128 * g
                    rcp[o:o + len(sel)] = rdeg[gd]
                    eTv[o:o + len(sel)] = _bf(ee[d][gp])
            # host-expanded step-0 streams
            src_glob = np.zeros(E_eff, np.int64)
            gsrc_glob = np.zeros(E_eff, np.int64)
            dst_loc = np.zeros(E_eff, np.int64)
            has_edge = np.zeros(E_eff, bool)
            for g in range(NCH):
                sel = np.asarray(groups[g], dtype=np.int64)
                o = g * G * 128
                if len(sel):
                    gp = perm[sel]
                    src_glob[o:o + len(sel)] = ei[d][0][gp]
                    gsrc_glob[o:o + len(sel)] = bt[d][ei[d][0][gp]]
                    dst_loc[o:o + len(sel)] = dl[sel]
                    has_edge[o:o + len(sel)] = True
            n0_ = node_start[d][c]
            xsv = np.where(has_edge[:, None], _bf(x[d][src_glob]), NPBF16(0))
            xdv = np.where(has_edge[:, None],
                           _bf(x[d][n0_ + dst_loc]), NPBF16(0))
            uev = np.zeros((E_eff, 128), NPBF16)
            if d == 0:
                u1c0 = np.concatenate([u_pad[0], u_pad[1]], 1)   # [512, 64]
                uev[:, 0:64] = np.where(has_edge[:, None], _bf(u1c0[gsrc_glob]),
                                        NPBF16(0))
            else:
                uev[:, 0:32] = np.where(has_edge[:, None], _bf(u_pad[1][gsrc_glob]),
                                        NPBF16(0))
            uev[:, 64:128] = eTv
            im[f'xsrcT{d}'] = np.ascontiguousarray(xsv.T)
            im[f'xdstT{d}'] = np.ascontiguousarray(xdv.T)
            im[f'ueT{d}'] = np.ascontiguousarray(uev.T)
            im[f'srcw{d}'] = _wrap_idx(e_src)
            im[f'dstw{d}'] = _wrap_idx(e_dstt)
            im[f'gsrcw{d}'] = _wrap_idx(e_gsrc)
            im[f'gsrcb{d}'] = np.ascontiguousarray(
                np.broadcast_to(e_gsrc[None, :].astype(np.float16), (128, E_eff)))
            im[f'dloc{d}'] = np.ascontiguousarray(dloc.reshape(NE_CH_PAD, 128).T.astype(np.float16))
            im[f'rcp{d}'] = np.ascontiguousarray(rcp.reshape(NE_CH_PAD, 128).T)
            n0, n1 = node_start[d][c], node_start[d][c + 1]
            bl = bt[d][n0:n1] - c * GPC
            Bnm = np.zeros((N_pad, GPC), NPBF16)
            Bnm[np.arange(n1 - n0), bl] = 1.0
            im[f'Bnm{d}'] = np.ascontiguousarray(
                Bnm.reshape(NCH, 128, GPC).transpose(1, 0, 2))
            im[f'Bt{d}'] = np.ascontiguousarray(Bnm.T)
            xt0 = np.zeros((64, N_pad), NPBF16)
            xt0[:, :n1 - n0] = _bf(x[d][n0:n1].T)
            im[f'xT{d}'] = xt0
            ut0 = np.ascontiguousarray(u_pad[d][c * GPC:(c + 1) * GPC].T)
            im[f'uT{d}'] = ut0
    return meta, in_maps


def _build(meta):
    N_pad, NCH, G = meta['N_pad'], meta['NCH'], meta['G']
    NE_CH, NE_CH_PAD, E_eff, NT = (meta['NE_CH'], meta['NE_CH_PAD'],
                                   meta['E_eff'], meta['NT'])
    node_tiles = []
    o = 0
    while o < N_pad:
        w = min(512, N_pad - o)
        node_tiles.append((o, w))
        o += w

    nc = bacc.Bacc("TRN2", target_bir_lowering=False, debug=False,
                   num_devices=NCORES, num_swdge_queues=4)
    DT = nc.dram_tensor
    ins = {}
    for nm, shp, dt in [
        ('we1a', [64, 128], BF16), ('we1b', [64, 128], BF16),
        ('we1ue', [128, 128], BF16), ('we1u1p', [64, 128], BF16),
        ('we1e', [64, 128], BF16), ('w1ua', [32, 128], BF16),
        ('w1ub', [32, 128], BF16), ('iotaG', [128, 4], F32),
        ('we2', [128, 128], BF16), ('we3', [128, 64], BF16),
        ('wn1x', [64, 128], BF16), ('wn1a', [64, 128], BF16),
        ('wn1ua', [32, 128], F32), ('wn1ub', [32, 128], F32),
        ('wn2', [128, 128], BF16), ('wn3', [128, 64], BF16),
        ('wa1', [64, 128], BF16), ('wa2', [128, 128], BF16), ('wa3', [128, 1], BF16),
        ('wg1p', [64, 128], F32), ('wg1a', [32, 128], F32), ('wg1b', [32, 128], F32),
        ('wg2', [128, 128], F32), ('wg3', [128, 32], F32),
        ('wo1a', [32, 128], F32), ('wo1b', [32, 128], F32),
        ('wo2', [128, 128], F32), ('wo3', [128, 2], F32),
        ('iota', [128, WIN], FP16), ('ident', [128, 128], F32),
        ('identb', [128, 128], BF16),
        ('wg1pb', [64, 128], BF16), ('wg1ab', [32, 128], BF16),
        ('wg1bb', [32, 128], BF16), ('wg2b', [128, 128], BF16),
        ('wg3b', [128, 32], BF16),
        ('xtbl0', [NCORES * N_pad, 128], BF16), ('xtbl1', [NCORES * N_pad, 128], BF16),
        ('utbl0', [GTOT, 128], BF16),
        ('uF0', [F_U, GTOT], BF16), ('uF1', [F_U, GTOT], BF16),
    ]:
        ins[nm] = DT(nm, shp, dt, kind="ExternalInput")
    for d in range(2):
        for nm, shp, dt in [
            (f'xsrcT{d}', [64, E_eff], BF16),
            (f'xdstT{d}', [64, E_eff], BF16),
            (f'ueT{d}', [128, E_eff], BF16),
            (f'srcw{d}', [128, E_eff // 16], I16),
            (f'dstw{d}', [128, E_eff // 16], I16),
            (f'gsrcw{d}', [128, E_eff // 16], I16),
            (f'gsrcb{d}', [128, E_eff], FP16),
            (f'dloc{d}', [128, NE_CH_PAD], FP16),
            (f'rcp{d}', [128, NE_CH_PAD], F32),
            (f'Bnm{d}', [128, NCH, GPC], BF16),
            (f'Bt{d}', [64, N_pad], BF16),
            (f'xT{d}', [64, N_pad], BF16),
            (f'uT{d}', [32, GPC], F32),
        ]:
            ins[nm] = DT(nm, shp, dt, kind="ExternalInput")
    out_d = DT("out", [N_STEPS, F_OUT, GPC], F32, kind="ExternalOutput")

    with tile.TileContext(nc) as tc:
        with (
            tc.tile_pool(name="const", bufs=1) as cpool,
            tc.tile_pool(name="pers", bufs=1) as pers,
            tc.tile_pool(name="gath", bufs=10) as gpool,
            tc.tile_pool(name="mlp", bufs=4) as mpool,
            tc.tile_pool(name="small", bufs=4) as spool,
            tc.tile_pool(name="ps_mlp", bufs=3, space="PSUM") as ps_mlp,
            tc.tile_pool(name="ps_l3", bufs=2, space="PSUM") as ps_l3,
            tc.tile_pool(name="ps_agg", bufs=2, space="PSUM") as ps_agg,
            tc.tile_pool(name="ps_pool", bufs=1, space="PSUM") as ps_pool,
            tc.tile_pool(name="dram", bufs=1, space="DRAM") as dpool,
        ):
            nc.gpsimd.load_library(library_config.mlp)

            def load_const(name):
                ap = ins[name]
                t = cpool.tile(list(ap.shape), ap.dtype, tag=name, name='c_' + name)
                nc.sync.dma_start(t[:], ap[:])
                return t

            W = {nm: load_const(nm) for nm in
                 ['we1a', 'we1b', 'we1ue', 'we1u1p', 'we2', 'we3', 'wn1x', 'wn1a',
                  'wn1ua', 'wn1ub', 'wn2', 'wn3',
                  'wa1', 'wa2', 'wa3', 'wg1p', 'wg1a', 'wg1b', 'wg2', 'wg3',
                  'wo1a', 'wo1b', 'wo2', 'wo3', 'iota', 'ident', 'identb',
                  'wg1pb', 'wg1ab', 'wg1bb', 'wg2b', 'wg3b',
                  'we1e', 'w1ua', 'w1ub', 'iotaG']}
            IDX = {}
            for d in range(2):
                for nm in [f'srcw{d}', f'dstw{d}', f'dloc{d}', f'rcp{d}',
                           f'Bnm{d}', f'Bt{d}']:
                    IDX[nm] = load_const(nm)

            xT = [[pers.tile([64, N_pad], BF16, tag=f'xT{d}_{s}',
                             name=f'xT{d}_{s}') for s in range(3)]
                  for d in range(2)]
            xnm = [pers.tile([128, NCH, 64], BF16, tag=f'xnm{d}', name=f'xnm{d}')
                   for d in range(2)]
            uT = [[pers.tile([32, GPC], F32, tag=f'uT{d}_{s}', name=f'uT{d}_{s}')
                   for s in range(3)]
                  for d in range(2)]
            uF = [[pers.tile([F_U, GTOT], BF16, tag=f'uF{d}_{k}', name=f'uF{d}_{k}')
                   for k in range(3)]
                  for d in range(2)]
            for d in range(2):
                nc.sync.dma_start(xT[d][0][:], ins[f'xT{d}'][:])
                nc.sync.dma_start(uT[d][0][:], ins[f'uT{d}'][:])
                nc.sync.dma_start(uF[d][0][:], ins[f'uF{d}'][:])

            eT_next = [dpool.tile([64, E_eff], BF16, tag=f'eTn{d}', name=f'eTn{d}')
                       for d in range(2)]
            xtbl_next = [dpool.tile([NCORES * N_pad, 128], BF16, tag=f'xtn{d}',
                                    name=f'xtn{d}')
                         for d in range(2)]
            utbl = {(0, 0): ins['utbl0']}
            for key in [(1, 0), (0, 1), (1, 1)]:
                utbl[key] = dpool.tile([GTOT, 128], BF16, tag=f'utbl{key[0]}{key[1]}',
                                       name=f'utbl{key[0]}{key[1]}')
            utbl_u1p = dpool.tile([GTOT, 128], BF16, tag='utblu1p', name='utblu1p')
            TuG = {k: pers.tile([128, 4, 128], BF16, tag=f'TuG{k[0]}{k[1]}',
                                name=f'TuG{k[0]}{k[1]}')
                   for k in [(1, 0), (0, 1), (1, 1)]}
            # boundary AllGathers: small pooled AG (urgent) + big x AG (lazy, s=0)
            agp = {}
            for key in [(0, 0), (1, 0), (0, 1)]:
                agp[key] = (dpool.tile([GPC, 64], BF16, name=f'agi{key[0]}{key[1]}',
                                       tag=f'agi{key[0]}{key[1]}'),
                            dpool.tile([GTOT, 64], BF16,
                                       name=f'ago{key[0]}{key[1]}',
                                       tag=f'ago{key[0]}{key[1]}'))
            agx = {d: (dpool.tile([N_pad, 64], BF16, name=f'agxi{d}', tag=f'agxi{d}'),
                       dpool.tile([NCORES * N_pad, 64], BF16, name=f'agxo{d}',
                                  tag=f'agxo{d}'))
                   for d in range(2)}

            RG = [list(range(NCORES))]

            def build_Tu(dest, parts):
                psT = ps_mlp.tile([128, 512], F32, tag='pmlp', space="PSUM",
                                  name='psT')
                for i, (wap, uap) in enumerate(parts):
                    nc.tensor.matmul(out=psT[:], lhsT=wap, rhs=uap,
                                     start=(i == 0), stop=(i == len(parts) - 1))
                TuT = spool.tile([128, 512], BF16, tag='TuT', bufs=2, name='TuT')
                nc.vector.tensor_copy(out=TuT[:], in_=psT[:])
                for a in range(4):
                    psxT = ps_pool.tile([128, GPC * 4], BF16, tag='ppool',
                                        space="PSUM", name='psxT')
                    nc.tensor.matmul(out=psxT[0:128, 0:128],
                                     lhsT=TuT[:, 128 * a:128 * a + 128],
                                     rhs=W['identb'][:], is_transpose=True,
                                     start=True, stop=True)
                    nc.vector.tensor_copy(out=dest[:, a, :],
                                          in_=psxT[0:128, 0:128])

            def repack_x(gout, table):
                nblk = NCORES * N_pad // 128
                half = nblk // 2
                rows = half * 128
                for hh in range(2):
                    bb = pers.tile([128, half, 64], BF16, tag='xbnc', name='xbnc')
                    nc.scalar.dma_start(
                        bb[:], gout[hh * rows:(hh + 1) * rows, :].rearrange(
                            "(c p) f -> p c f", p=128))
                    nc.scalar.dma_start(
                        table[hh * rows:(hh + 1) * rows, 0:64].rearrange(
                            "(c p) f -> p c f", p=128), bb[:])

            def transpose_32x512_to_gm(src, psdst_tag, identb):
                # src [32, 512] bf16 -> returns sbuf [128, 4, 32] bf16 graph-major
                outt = spool.tile([128, 4, 32], BF16, tag='ugm2', bufs=2, name='ugm2')
                for a in range(4):
                    pstx = ps_pool.tile([128, GPC * 4], BF16, tag='ppool', space="PSUM",
                                        name='pstx')
                    nc.tensor.matmul(out=pstx[0:128, 0:32],
                                     lhsT=src[:, 128 * a:128 * a + 128],
                                     rhs=identb[0:32, 0:32], is_transpose=True,
                                     start=True, stop=True)
                    nc.vector.tensor_copy(out=outt[:, a, :], in_=pstx[0:128, 0:32])
                return outt

            def build_utable(table, u_own, u_other, identb):
                for col0, src in ((0, u_own), (32, u_other)):
                    gm = transpose_32x512_to_gm(src, 'pstx', identb)
                    nc.sync.dma_start(
                        table[:, col0:col0 + 32].rearrange("(c p) f -> p c f", p=128),
                        gm[:])

            def gnn_step(d, s):
                xtbl_cur = ins[f'xtbl{d}'] if s == 0 else xtbl_next[d]
                eT_cur = eT_next[d]
                utbl_cur = utbl[(d, s)]
                srcw, dstw = IDX[f'srcw{d}'], IDX[f'dstw{d}']
                dlocs, rcps = IDX[f'dloc{d}'], IDX[f'rcp{d}']

                drains = pers.tile([64, NCH, WIN], BF16, tag='drains')
                aggT = pers.tile([64, N_pad], BF16, tag='aggT')
                psa_ref = [None]
                qrr = [0]

                def u_term_mms(ps1, tky, ecol):
                    gsb = gpool.tile([128, 512], FP16, tag='gsb', bufs=4, name='gsb')
                    nc.scalar.dma_start(gsb[:], ins[f'gsrcb{d}'][:, ecol])
                    for kt in range(4):
                        ohg = spool.tile([128, 512], BF16, tag='ohg', bufs=6,
                                         name='ohg')
                        nc.vector.tensor_scalar(out=ohg[:], in0=gsb[:],
                                                scalar1=W['iotaG'][:, kt:kt + 1],
                                                scalar2=None, op0=ALU.is_equal)
                        nc.tensor.matmul(out=ps1[:], lhsT=TuG[tky][:, kt, :],
                                         rhs=ohg[:], start=False, stop=(kt == 3))

                KPF = 10
                xg = {}

                def emit_xgather(t):
                    gsrc = gpool.tile([128, 1, 512], BF16, tag='gsrc', bufs=KPF + 2,
                                      name='gsrc')
                    gdst = gpool.tile([128, 1, 512], BF16, tag='gdst', bufs=KPF + 2,
                                      name='gdst')
                    ic = slice(32 * t, 32 * t + 32)
                    q = qrr[0]; qrr[0] = (q + 1) % 4
                    nc.gpsimd.dma_gather(gsrc[:], xtbl_cur[:], srcw[:, ic], 512, 512,
                                         128, transpose=True, queue_num=q)
                    q = qrr[0]; qrr[0] = (q + 1) % 4
                    nc.gpsimd.dma_gather(gdst[:], xtbl_cur[:], dstw[:, ic], 512, 512,
                                         128, transpose=True, queue_num=q)
                    xg[t] = (gsrc, gdst)

                if s == 1:
                    for t in range(min(KPF, NT)):
                        emit_xgather(t)
                for t in range(NT):
                    es = slice(0, 512)
                    ecol = slice(512 * t, 512 * t + 512)
                    ic = slice(32 * t, 32 * t + 32)
                    ps1 = ps_mlp.tile([128, 512], F32, tag='pmlp', space="PSUM")
                    if s == 0:
                        xs = gpool.tile([64, 512], BF16, tag='xs', bufs=4, name='xs')
                        xd = gpool.tile([64, 512], BF16, tag='xd', bufs=4, name='xd')
                        gue = gpool.tile([128, 512], BF16, tag='gue', bufs=4,
                                         name='gue')
                        nc.scalar.dma_start(xs[:], ins[f'xsrcT{d}'][:, ecol])
                        nc.scalar.dma_start(xd[:], ins[f'xdstT{d}'][:, ecol])
                        nc.sync.dma_start(gue[:], ins[f'ueT{d}'][:, ecol])
                        nc.tensor.matmul(out=ps1[:], lhsT=W['we1a'][:], rhs=xs[:],
                                         start=True, stop=False)
                        nc.tensor.matmul(out=ps1[:], lhsT=W['we1b'][:], rhs=xd[:],
                                         start=False, stop=False)
                        nc.tensor.matmul(out=ps1[:], lhsT=W['we1ue'][:], rhs=gue[:],
                                         start=False, stop=(d == 0))
                        if d == 1:
                            u_term_mms(ps1, (1, 0), ecol)
                    else:
                        gsrc, gdst = xg.pop(t)
                        if t + KPF < NT:
                            emit_xgather(t + KPF)
                        et = gpool.tile([64, 512], BF16, tag='et', bufs=4, name='et')
                        nc.sync.dma_start(et[:], eT_cur[:, ecol])
                        nc.tensor.matmul(out=ps1[:], lhsT=W['we1a'][:],
                                         rhs=gsrc[0:64, 0, es], start=True, stop=False)
                        nc.tensor.matmul(out=ps1[:], lhsT=W['we1b'][:],
                                         rhs=gdst[0:64, 0, es], start=False, stop=False)
                        nc.tensor.matmul(out=ps1[:], lhsT=W['we1e'][:],
                                         rhs=et[:], start=False, stop=False)
                        u_term_mms(ps1, (d, 1), ecol)
                    if True:
                      if True:
                        pass
                        h1 = mpool.tile([128, 512], BF16, tag='h1')
                        nc.scalar.activation(out=h1[:], in_=ps1[:], func=ACT.Relu)
                        ps2 = ps_mlp.tile([128, 512], F32, tag='pmlp', space="PSUM")
                        nc.tensor.matmul(out=ps2[:], lhsT=W['we2'][:], rhs=h1[:],
                                         start=True, stop=True)
                        h2 = mpool.tile([128, 512], BF16, tag='h2')
                        nc.scalar.activation(out=h2[:], in_=ps2[:], func=ACT.Relu)
                        if s == 0:
                            ps3 = ps_l3.tile([128, 512], F32, tag='pl3', space="PSUM")
                            nc.tensor.matmul(out=ps3[0:64, :], lhsT=W['we3'][:],
                                             rhs=h2[:], start=True, stop=True)
                            en = mpool.tile([64, 512], BF16, tag='en', bufs=2)
                            nc.vector.tensor_copy(out=en[:], in_=ps3[0:64, :])
                            nc.sync.dma_start(eT_next[d][:, 512 * t:512 * t + 512],
                                              en[:])
                        psd = ps_l3.tile([128, 512], F32, tag='pl3', space="PSUM")
                        for j in range(4):
                            if 4 * t + j >= NE_CH:
                                continue
                            nc.tensor.matmul(out=psd[:, 64 * j:64 * j + 64],
                                             lhsT=h2[:, 128 * j:128 * j + 128],
                                             rhs=W['we3'][:], start=True, stop=True)
                        eema = spool.tile([128, 4, 64], BF16, tag='eema', bufs=4,
                                          name='eema')
                        nc.vector.tensor_tensor(
                            out=eema[:],
                            in0=psd[:, 0:256].rearrange("p (c f) -> p c f", f=64),
                            in1=rcps[:, 4 * t:4 * t + 4, None].to_broadcast(
                                [128, 4, 64]),
                            op=ALU.mult)
                        for j in range(4):
                            k = 4 * t + j
                            if k >= NE_CH:
                                continue
                            g = k // G
                            w_in = k % G
                            oh = spool.tile([128, WIN], BF16, tag='oh', bufs=8)
                            nc.vector.tensor_tensor(
                                out=oh[:], in0=W['iota'][:],
                                in1=dlocs[:, k:k + 1].to_broadcast([128, WIN]),
                                op=ALU.is_equal)
                            if w_in == 0:
                                psa_ref[0] = ps_agg.tile([64, WIN], F32, tag='pagg',
                                                         space="PSUM", name='psa')
                            psa = psa_ref[0]
                            nc.tensor.matmul(out=psa[:], lhsT=eema[:, j, :], rhs=oh[:],
                                             start=(w_in == 0), stop=(w_in == G - 1))
                            if w_in == G - 1:
                                nc.vector.tensor_copy(out=drains[:, g, :], in_=psa[:])
                for cch in range(NCH):
                    if cch == 0:
                        nc.vector.tensor_copy(out=aggT[:, 0:128],
                                              in_=drains[:, 0, 0:128])
                    else:
                        nc.vector.tensor_add(out=aggT[:, 128 * cch:128 * cch + 128],
                                             in0=drains[:, cch, 0:128],
                                             in1=drains[:, cch - 1, 128:256])

                # UW[g, H] = u1c_gm[g] @ Wn1u, built as two K=32 matmuls
                psuw = ps_pool.tile([128, GPC * 2], F32, tag='ppool', space="PSUM")
                nc.tensor.matmul(out=psuw[0:GPC, 0:128], lhsT=uT[d][s][:],
                                 rhs=W['wn1ua'][:], start=True, stop=False)
                nc.tensor.matmul(out=psuw[0:GPC, 0:128], lhsT=uT[1 - d][s + d][:],
                                 rhs=W['wn1ub'][:], start=False, stop=True)
                UWb = spool.tile([GPC, 128], BF16, tag='uwb')
                nc.vector.tensor_copy(out=UWb[:], in_=psuw[0:GPC, 0:128])

                w_nm = pers.tile([128, NCH], F32, tag='w_nm')
                psP = ps_pool.tile([65, GPC], F32, tag='ppool', space="PSUM")
                for (off, wdt) in node_tiles:
                    sl = slice(off, off + wdt)
                    psn = ps_mlp.tile([128, 512], F32, tag='pmlp', space="PSUM")
                    nc.tensor.matmul(out=psn[:, :wdt], lhsT=W['wn1x'][:],
                                     rhs=xT[d][s][:, sl], start=True, stop=False)
                    nc.tensor.matmul(out=psn[:, :wdt], lhsT=W['wn1a'][:],
                                     rhs=aggT[:, sl], start=False, stop=False)
                    nc.tensor.matmul(out=psn[:, :wdt], lhsT=UWb[:],
                                     rhs=IDX[f'Bt{d}'][:, sl], start=False, stop=True)
                    nh1 = mpool.tile([128, 512], BF16, tag='h1')
                    nc.scalar.activation(out=nh1[:, :wdt], in_=psn[:, :wdt],
                                         func=ACT.Relu)
                    psn2 = ps_mlp.tile([128, 512], F32, tag='pmlp', space="PSUM")
                    nc.tensor.matmul(out=psn2[:, :wdt], lhsT=W['wn2'][:],
                                     rhs=nh1[:, :wdt], start=True, stop=True)
                    nh2 = mpool.tile([128, 512], BF16, tag='h2')
                    nc.scalar.activation(out=nh2[:, :wdt], in_=psn2[:, :wdt],
                                         func=ACT.Relu)
                    psx = ps_l3.tile([128, 512], F32, tag='pl3', space="PSUM")
                    nc.tensor.matmul(out=psx[0:64, :wdt], lhsT=W['wn3'][:],
                                     rhs=nh2[:, :wdt], start=True, stop=True)
                    nc.vector.tensor_copy(out=xT[d][s + 1][:, sl], in_=psx[0:64, :wdt])
                    nch_here = wdt // 128
                    for j in range(nch_here):
                        cch = off // 128 + j
                        psd = ps_l3.tile([128, 512], F32, tag='pl3', space="PSUM")
                        nc.tensor.matmul(out=psd[:, 0:64],
                                         lhsT=nh2[:, 128 * j:128 * j + 128],
                                         rhs=W['wn3'][:], start=True, stop=True)
                        nc.vector.tensor_copy(out=xnm[d][:, cch, :], in_=psd[:, 0:64])
                    psa1 = ps_mlp.tile([128, 512], F32, tag='pmlp', space="PSUM")
                    nc.tensor.matmul(out=psa1[:, :wdt], lhsT=W['wa1'][:],
                                     rhs=xT[d][s + 1][:, sl], start=True, stop=True)
                    ah1 = mpool.tile([128, 512], BF16, tag='h1')
                    nc.scalar.activation(out=ah1[:, :wdt], in_=psa1[:, :wdt],
                                         func=ACT.Relu)
                    psa2 = ps_mlp.tile([128, 512], F32, tag='pmlp', space="PSUM")
                    nc.tensor.matmul(out=psa2[:, :wdt], lhsT=W['wa2'][:],
                                     rhs=ah1[:, :wdt], start=True, stop=True)
                    ah2 = mpool.tile([128, 512], BF16, tag='h2')
                    nc.scalar.activation(out=ah2[:, :wdt], in_=psa2[:, :wdt],
                                         func=ACT.Relu)
                    for j in range(nch_here):
                        cch = off // 128 + j
                        pss = ps_l3.tile([128, 512], F32, tag='pl3', space="PSUM")
                        nc.tensor.matmul(out=pss[:, 0:1],
                                         lhsT=ah2[:, 128 * j:128 * j + 128],
                                         rhs=W['wa3'][:], start=True, stop=True)
                        nc.scalar.activation(out=w_nm[:, cch:cch + 1], in_=pss[:, 0:1],
                                             func=ACT.Exp)
                        pw = spool.tile([128, 65], BF16, tag='pw')
                        nc.vector.tensor_scalar(out=pw[:, 0:64],
                                                in0=xnm[d][:, cch, :],
                                                scalar1=w_nm[:, cch:cch + 1],
                                                scalar2=None, op0=ALU.mult)
                        nc.vector.tensor_copy(out=pw[:, 64:65],
                                              in_=w_nm[:, cch:cch + 1])
                        nc.tensor.matmul(out=psP[:], lhsT=pw[:],
                                         rhs=IDX[f'Bnm{d}'][:, cch, :],
                                         start=(cch == 0), stop=(cch == NCH - 1))

                pooln = spool.tile([65, GPC], F32, tag='pooln')
                nc.vector.tensor_copy(out=pooln[:], in_=psP[:])
                pst = ps_pool.tile([128, GPC * 2], F32, tag='ppool', space="PSUM")
                nc.tensor.matmul(out=pst[0:GPC, 0:65], lhsT=pooln[:],
                                 rhs=W['ident'][0:65, 0:65],
                                 is_transpose=True, start=True, stop=True)
                pgm = spool.tile([GPC, 65], F32, tag='pgm')
                nc.vector.tensor_copy(out=pgm[:], in_=pst[0:GPC, 0:65])
                dn = spool.tile([GPC, 1], F32, tag='dn')
                nc.vector.tensor_scalar(out=dn[:], in0=pgm[:, 64:65], scalar1=1e-16,
                                        scalar2=None, op0=ALU.max)
                rdn = spool.tile([GPC, 1], F32, tag='rdn')
                nc.vector.reciprocal(out=rdn[:], in_=dn[:])
                pgs = spool.tile([GPC, 64], F32, tag='pgs')
                nc.vector.tensor_scalar(out=pgs[:], in0=pgm[:, 0:64], scalar1=rdn[:],
                                        scalar2=None, op0=ALU.mult)
                psb = ps_pool.tile([128, GPC * 2], F32, tag='ppool', space="PSUM")
                nc.tensor.matmul(out=psb[0:64, 0:GPC], lhsT=pgs[:],
                                 rhs=W['ident'][0:GPC, 0:GPC],
                                 is_transpose=True, start=True, stop=True)
                pooledT = spool.tile([64, GPC], F32, tag='pooledT')
                nc.vector.tensor_copy(out=pooledT[:], in_=psb[0:64, 0:GPC])
                psg = ps_pool.tile([128, GPC * 2], F32, tag='ppool', space="PSUM")
                nc.tensor.matmul(out=psg[:, 0:GPC], lhsT=W['wg1p'][:], rhs=pooledT[:],
                                 start=True, stop=False)
                nc.tensor.matmul(out=psg[:, 0:GPC], lhsT=W['wg1a'][:], rhs=uT[d][s][:],
                                 start=False, stop=False)
                nc.tensor.matmul(out=psg[:, 0:GPC], lhsT=W['wg1b'][:],
                                 rhs=uT[1 - d][s + d][:], start=False, stop=True)
                gh1 = spool.tile([128, GPC], F32, tag='gh1')
                nc.scalar.activation(out=gh1[:], in_=psg[:, 0:GPC], func=ACT.Relu)
                psg2 = ps_pool.tile([128, GPC * 2], F32, tag='ppool', space="PSUM")
                nc.tensor.matmul(out=psg2[:, 0:GPC], lhsT=W['wg2'][:], rhs=gh1[:],
                                 start=True, stop=True)
                gh2 = spool.tile([128, GPC], F32, tag='gh2')
                nc.scalar.activation(out=gh2[:], in_=psg2[:, 0:GPC], func=ACT.Relu)
                psg3 = ps_pool.tile([128, GPC * 2], F32, tag='ppool', space="PSUM")
                nc.tensor.matmul(out=psg3[0:32, 0:GPC], lhsT=W['wg3'][:], rhs=gh2[:],
                                 start=True, stop=True)
                nc.vector.tensor_copy(out=uT[d][s + 1][:], in_=psg3[0:32, 0:GPC])

                # ---- boundary: small pooled AG first, big x AG after ----
                if (d, s) != (1, 1):
                    pgsb = spool.tile([GPC, 64], BF16, tag='pgsb')
                    nc.vector.tensor_copy(out=pgsb[:], in_=pgs[:])
                    gin_, gout_ = agp[(d, s)]
                    nc.sync.dma_start(gin_[:], pgsb[:])
                    nc.gpsimd.collective_compute(
                        "AllGather", ALU.bypass, replica_groups=RG,
                        ins=[gin_[:].opt()], outs=[gout_[:].opt()])
                    if s == 0:
                        xgi, xgo = agx[d]
                        nc.scalar.dma_start(
                            xgi[:].rearrange("(c p) f -> p c f", p=128), xnm[d][:])
                        nc.gpsimd.collective_compute(
                            "AllGather", ALU.bypass, replica_groups=RG,
                            ins=[xgi[:].opt()], outs=[xgo[:].opt()])
                        repack_x(xgo, xtbl_next[d])
                    # pooled for all graphs -> sbuf graph-major [64, 8, 64]
                    pAll = spool.tile([GPC, NCORES, 64], BF16, tag='pAll', bufs=2)
                    for a in range(NCORES):
                        nc.sync.dma_start(pAll[:, a, :],
                                          gout_[a * GPC:(a + 1) * GPC, :])
                    # pooledT_full [64 f, 512 g]
                    pTf = spool.tile([64, GTOT], BF16, tag='pTf', bufs=2)
                    for a in range(NCORES):
                        pstp = ps_pool.tile([128, GPC * 4], BF16, tag='ppool',
                                            space="PSUM", name='pstp')
                        nc.tensor.matmul(out=pstp[0:64, 0:GPC], lhsT=pAll[:, a, :],
                                         rhs=W['identb'][0:GPC, 0:GPC],
                                         is_transpose=True, start=True, stop=True)
                        nc.vector.tensor_copy(out=pTf[:, GPC * a:GPC * (a + 1)],
                                              in_=pstp[0:64, 0:GPC])
                    # replicated glob MLP (bf16): uF[d][s+1] for ALL graphs
                    uFA = uF[d][s]
                    uFB = uF[1 - d][s + d]
                    psG = ps_mlp.tile([128, 512], F32, tag='pmlp', space="PSUM",
                                      name='psG')
                    nc.tensor.matmul(out=psG[:], lhsT=W['wg1pb'][:], rhs=pTf[:],
                                     start=True, stop=False)
                    nc.tensor.matmul(out=psG[:], lhsT=W['wg1ab'][:], rhs=uFA[:],
                                     start=False, stop=False)
                    nc.tensor.matmul(out=psG[:], lhsT=W['wg1bb'][:], rhs=uFB[:],
                                     start=False, stop=True)
                    gH1 = mpool.tile([128, 512], BF16, tag='h1', name='gH1')
                    nc.scalar.activation(out=gH1[:], in_=psG[:], func=ACT.Relu)
                    psG2 = ps_mlp.tile([128, 512], F32, tag='pmlp', space="PSUM",
                                       name='psG2')
                    nc.tensor.matmul(out=psG2[:], lhsT=W['wg2b'][:], rhs=gH1[:],
                                     start=True, stop=True)
                    gH2 = mpool.tile([128, 512], BF16, tag='h2', name='gH2')
                    nc.scalar.activation(out=gH2[:], in_=psG2[:], func=ACT.Relu)
                    psG3 = ps_l3.tile([128, 512], F32, tag='pl3', space="PSUM",
                                      name='psG3')
                    nc.tensor.matmul(out=psG3[0:F_U, :], lhsT=W['wg3b'][:], rhs=gH2[:],
                                     start=True, stop=True)
                    nc.vector.tensor_copy(out=uF[d][s + 1][:], in_=psG3[0:F_U, :])
                    # build the u-table for the NEXT gnn_step
                    nxt = {(0, 0): (1, 0), (1, 0): (0, 1), (0, 1): (1, 1)}[(d, s)]
                    if (d, s) == (0, 0):
                        build_Tu(TuG[(1, 0)], [(W['w1ub'][:], uF[0][1][:])])
                    elif (d, s) == (1, 0):
                        build_Tu(TuG[(0, 1)], [(W['w1ua'][:], uF[0][1][:]),
                                               (W['w1ub'][:], uF[1][1][:])])
                    else:
                        build_Tu(TuG[(1, 1)], [(W['w1ua'][:], uF[1][1][:]),
                                               (W['w1ub'][:], uF[0][2][:])])

            for s in range(N_STEPS):
                for d in range(2):
                    gnn_step(d, s)
                pso = ps_pool.tile([128, GPC * 2], F32, tag='ppool', space="PSUM")
                nc.tensor.matmul(out=pso[:, 0:GPC], lhsT=W['wo1a'][:],
                                 rhs=uT[0][s + 1][:], start=True, stop=False)
                nc.tensor.matmul(out=pso[:, 0:GPC], lhsT=W['wo1b'][:],
                                 rhs=uT[1][s + 1][:], start=False, stop=True)
                oh1 = spool.tile([128, GPC], F32, tag='oh1')
                nc.scalar.activation(out=oh1[:], in_=pso[:, 0:GPC], func=ACT.Relu)
                pso2 = ps_pool.tile([128, GPC * 2], F32, tag='ppool', space="PSUM")
                nc.tensor.matmul(out=pso2[:, 0:GPC], lhsT=W['wo2'][:], rhs=oh1[:],
                                 start=True, stop=True)
                oh2 = spool.tile([128, GPC], F32, tag='oh2')
                nc.scalar.activation(out=oh2[:], in_=pso2[:, 0:GPC], func=ACT.Relu)
                pso3 = ps_pool.tile([128, GPC * 2], F32, tag='ppool', space="PSUM")
                nc.tensor.matmul(out=pso3[0:F_OUT, 0:GPC], lhsT=W['wo3'][:], rhs=oh2[:],
                                 start=True, stop=True)
                ot = spool.tile([F_OUT, GPC], F32, tag='ot')
                nc.vector.tensor_copy(out=ot[:], in_=pso3[0:F_OUT, 0:GPC])
                nc.sync.dma_start(out_d[s], ot[:])
    nc.compile()
    return nc


_CACHE = {}


def _get_nc(meta):
    key = tuple(sorted(meta.items()))
    if key not in _CACHE:
        _CACHE[key] = _build(meta)
    return _CACHE[key]


def kernel(**inputs):
    meta, in_maps = _prep(inputs)
    nc = _get_nc(meta)
    res = run_bass_kernel_spmd(nc, in_maps, core_ids=list(range(NCORES)))
    out = np.zeros((N_STEPS, B, F_OUT), np.float32)
    for c in range(NCORES):
        o = res.results[c]["out"]
        g0 = c * GPC
        g1 = min(B, g0 + GPC)
        if g1 > g0:
            out[:, g0:g1, :] = np.transpose(o, (0, 2, 1))[:, :g1 - g0, :]
    return out


# revision 28
# speedup vs baseline: 1.1496x; 1.1496x over previous
"""Trainium2 Bass kernel for nn_AlternatingSimple (GNN message passing).

Self-contained: takes full (unsharded) numpy inputs, shards across 8
NeuronCores (data-parallel over graphs), runs one SPMD Bass/Tile NEFF,
gathers the full output.
"""
import sys, os
sys.path.insert(0, '/opt/trn_rl_repo')
import numpy as np
import ml_dtypes

import concourse.bass as bass
import concourse.bacc as bacc
import concourse.tile as tile
import concourse.mybir as mybir
from concourse import library_config
from concourse.bass_utils import run_bass_kernel_spmd

BF16 = mybir.dt.bfloat16
FP16 = mybir.dt.float16
F32 = mybir.dt.float32
I16 = mybir.dt.int16
NPBF16 = ml_dtypes.bfloat16

NCORES = 8
B = 500
GPC = 64          # graphs per core (padded: 8*64 = 512 >= 500)
GTOT = NCORES * GPC
F_X = 64
F_U = 32
F_OUT = 2
N_STEPS = 2
WIN = 256          # aggregation node window (2 chunks of 128)

ACT = mybir.ActivationFunctionType
ALU = mybir.AluOpType


def _wrap_idx(a):
    """int16 index array -> [128, ceil(n/16)] wrapped in 16 partitions, replicated 8x."""
    n = len(a)
    ncol = (n + 15) // 16
    w = np.zeros((16, ncol), np.int16)
    w[np.arange(n) % 16, np.arange(n) // 16] = a
    return np.tile(w, (8, 1))


def _bf(a):
    return np.asarray(a, np.float32).astype(NPBF16)


def _group_edges(dst_local, n_chunks, G):
    """Assign dst-sorted edges to n_chunks groups of capacity G*128; group g may
    only hold edges with dst_local in [128g, 128g+WIN). Returns groups or None."""
    cap = G * 128
    groups = [[] for _ in range(n_chunks)]
    chunk_of = dst_local // 128
    order = np.arange(len(dst_local))
    for c in range(n_chunks):
        idxs = order[chunk_of == c]
        pos = 0
        if c >= 1:
            spare = cap - len(groups[c - 1])
            take = min(spare, len(idxs))
            groups[c - 1].extend(idxs[:take])
            pos = take
        rest = idxs[pos:]
        if len(groups[c]) + len(rest) > cap:
            return None
        groups[c].extend(rest)
    return groups


def _prep(inputs):
    """Host-side sharding and constant packing. Returns (meta, in_maps)."""
    gi = lambda k: np.asarray(inputs[k])
    x = [gi('x1').astype(np.float32), gi('x2').astype(np.float32)]
    ei = [gi('edge_index1').astype(np.int64), gi('edge_index2').astype(np.int64)]
    ee = [gi('e1').astype(np.float32), gi('e2').astype(np.float32)]
    uu = [gi('u1').astype(np.float32), gi('u2').astype(np.float32)]
    bt = [gi('batch1').astype(np.int64), gi('batch2').astype(np.int64)]

    params = {}
    for nm in ['params_edge', 'params_node', 'params_attn', 'params_glob', 'params_out']:
        Ws = []
        for (Wt, bb) in inputs[nm]:
            Ws.append(np.asarray(Wt, np.float32))
            assert np.allclose(np.asarray(bb), 0.0), f"nonzero bias in {nm}"
        params[nm] = Ws

    node_start = np.zeros((2, NCORES + 1), np.int64)
    for d in range(2):
        node_start[d] = np.searchsorted(bt[d], np.arange(NCORES + 1) * GPC)
    ncnt = node_start[:, 1:] - node_start[:, :-1]
    N_pad = int(np.ceil(ncnt.max() / 128) * 128)
    assert NCORES * N_pad < 32768
    NCH = N_pad // 128

    shards = [[None] * NCORES for _ in range(2)]
    for d in range(2):
        src_g, dst_g = ei[d][0], ei[d][1]
        core_of_dst = bt[d][dst_g] // GPC
        for c in range(NCORES):
            m = np.nonzero(core_of_dst == c)[0]
            dl = (dst_g[m] - node_start[d][c]).astype(np.int64)
            o = np.argsort(dl, kind='stable')
            shards[d][c] = (m[o], dl[o])

    G = max(2, int(np.ceil(max(len(s[0]) for dd in shards for s in dd) / (128 * NCH))))
    while True:
        if all(_group_edges(shards[d][c][1], NCH, G) is not None
               for d in range(2) for c in range(NCORES)):
            break
        G += 1
    NE_CH = NCH * G
    NE_CH_PAD = ((NE_CH + 3) // 4) * 4
    E_eff = NE_CH_PAD * 128
    NT = E_eff // 512

    meta = dict(N_pad=N_pad, NCH=NCH, G=G, NE_CH=NE_CH, NE_CH_PAD=NE_CH_PAD,
                E_eff=E_eff, NT=NT)

    in_maps = [dict() for _ in range(NCORES)]

    We = params['params_edge']; Wn = params['params_node']
    Wa = params['params_attn']; Wg = params['params_glob']; Wo = params['params_out']
    shared = {
        'we1a': _bf(We[0][0:64]), 'we1b': _bf(We[0][64:128]),
        'we1u1p': _bf(np.concatenate([We[0][224:256], np.zeros((32, 128), np.float32)], 0)),
        'we1ue': _bf(np.concatenate([We[0][192:256], We[0][128:192]], 0)),
        'we1e': _bf(We[0][128:192]),
        'w1ua': _bf(We[0][192:224]), 'w1ub': _bf(We[0][224:256]),
        'we2': _bf(We[1]), 'we3': _bf(We[2]),
        'wn1x': _bf(Wn[0][0:64]), 'wn1a': _bf(Wn[0][64:128]),
        'wn1ua': np.asarray(Wn[0][128:160], np.float32),
        'wn1ub': np.asarray(Wn[0][160:192], np.float32),
        'wn2': _bf(Wn[1]), 'wn3': _bf(Wn[2]),
        'wa1': _bf(Wa[0]), 'wa2': _bf(Wa[1]), 'wa3': _bf(Wa[2]),
        'wg1p': np.asarray(Wg[0][0:64], np.float32),
        'wg1a': np.asarray(Wg[0][64:96], np.float32),
        'wg1b': np.asarray(Wg[0][96:128], np.float32),
        'wg2': np.asarray(Wg[1], np.float32), 'wg3': np.asarray(Wg[2], np.float32),
        'wo1a': np.asarray(Wo[0][0:32], np.float32),
        'wo1b': np.asarray(Wo[0][32:64], np.float32),
        'wo2': np.asarray(Wo[1], np.float32), 'wo3': np.asarray(Wo[2], np.float32),
        'iota': np.broadcast_to(np.arange(WIN, dtype=np.float16), (128, WIN)).copy(),
        'ident': np.eye(128, dtype=np.float32),
        'identb': np.eye(128, dtype=np.float32).astype(NPBF16),
        'iotaG': (np.arange(128)[:, None] + 128 * np.arange(4)[None, :]).astype(np.float32),
        'wg1pb': _bf(Wg[0][0:64]), 'wg1ab': _bf(Wg[0][64:96]),
        'wg1bb': _bf(Wg[0][96:128]), 'wg2b': _bf(Wg[1]), 'wg3b': _bf(Wg[2]),
    }

    for d in range(2):
        tblx = np.zeros((NCORES * N_pad, 128), NPBF16)
        for c in range(NCORES):
            n0, n1 = node_start[d][c], node_start[d][c + 1]
            tblx[c * N_pad: c * N_pad + (n1 - n0), 0:64] = _bf(x[d][n0:n1])
        shared[f'xtbl{d}'] = tblx

    u_pad = [np.zeros((GTOT, F_U), np.float32) for _ in range(2)]
    for d in range(2):
        u_pad[d][:B] = uu[d]
    tblu0 = np.zeros((GTOT, 128), NPBF16)
    tblu0[:, 0:32] = _bf(u_pad[0]); tblu0[:, 32:64] = _bf(u_pad[1])
    shared['utbl0'] = tblu0
    shared['uF0'] = _bf(u_pad[0].T)
    shared['uF1'] = _bf(u_pad[1].T)

    def tbl_row(d, nodes):
        cor = bt[d][nodes] // GPC
        return (cor * N_pad + nodes - node_start[d][cor]).astype(np.int16)

    for c in range(NCORES):
        im = in_maps[c]
        im.update(shared)
        for d in range(2):
            perm, dl = shards[d][c]
            groups = _group_edges(dl, NCH, G)
            e_src = np.zeros(E_eff, np.int16)
            e_dstt = np.zeros(E_eff, np.int16)
            e_gsrc = np.zeros(E_eff, np.int16)
            dloc = np.full(E_eff, -1.0, np.float32)
            rcp = np.zeros(E_eff, np.float32)
            eTv = np.zeros((E_eff, 64), NPBF16)
            deg = np.bincount(dl, minlength=N_pad).astype(np.float32)
            rdeg = 1.0 / np.maximum(deg, 1.0)
            for g in range(NCH):
                sel = np.asarray(groups[g], dtype=np.int64)
                o = g * G * 128
                if len(sel):
                    gp = perm[sel]
                    gd = dl[sel]
                    e_src[o:o + len(sel)] = tbl_row(d, ei[d][0][gp])
                    e_dstt[o:o + len(sel)] = (c * N_pad + gd).astype(np.int16)
                    e_gsrc[o:o + len(sel)] = bt[d][ei[d][0][gp]].astype(np.int16)
                    dloc[o:o + len(sel)] = gd - 128 * g
                    rcp[o:o + len(sel)] = rdeg[gd]
                    eTv[o:o + len(sel)] = _bf(ee[d][gp])
            # host-expanded step-0 streams
            src_glob = np.zeros(E_eff, np.int64)
            gsrc_glob = np.zeros(E_eff, np.int64)
            dst_loc = np.zeros(E_eff, np.int64)
            has_edge = np.zeros(E_eff, bool)
            for g in range(NCH):
                sel = np.asarray(groups[g], dtype=np.int64)
                o = g * G * 128
                if len(sel):
                    gp = perm[sel]
                    src_glob[o:o + len(sel)] = ei[d][0][gp]
                    gsrc_glob[o:o + len(sel)] = bt[d][ei[d][0][gp]]
                    dst_loc[o:o + len(sel)] = dl[sel]
                    has_edge[o:o + len(sel)] = True
            n0_ = node_start[d][c]
            xsv = np.where(has_edge[:, None], _bf(x[d][src_glob]), NPBF16(0))
            xdv = np.where(has_edge[:, None],
                           _bf(x[d][n0_ + dst_loc]), NPBF16(0))
            uev = np.zeros((E_eff, 128), NPBF16)
            if d == 0:
                u1c0 = np.concatenate([u_pad[0], u_pad[1]], 1)   # [512, 64]
                uev[:, 0:64] = np.where(has_edge[:, None], _bf(u1c0[gsrc_glob]),
                                        NPBF16(0))
            else:
                uev[:, 0:32] = np.where(has_edge[:, None], _bf(u_pad[1][gsrc_glob]),
                                        NPBF16(0))
            uev[:, 64:128] = eTv
            im[f'xsrcT{d}'] = np.ascontiguousarray(xsv.T)
            im[f'xdstT{d}'] = np.ascontiguousarray(xdv.T)
            im[f'ueT{d}'] = np.ascontiguousarray(uev.T)
            im[f'srcw{d}'] = _wrap_idx(e_src)
            im[f'dstw{d}'] = _wrap_idx(e_dstt)
            im[f'gsrcw{d}'] = _wrap_idx(e_gsrc)
            im[f'gsrcb{d}'] = np.ascontiguousarray(
                np.broadcast_to(e_gsrc[None, :].astype(np.float16), (128, E_eff)))
            im[f'dloc{d}'] = np.ascontiguousarray(dloc.reshape(NE_CH_PAD, 128).T.astype(np.float16))
            im[f'rcp{d}'] = np.ascontiguousarray(rcp.reshape(NE_CH_PAD, 128).T)
            n0, n1 = node_start[d][c], node_start[d][c + 1]
            bl = bt[d][n0:n1] - c * GPC
            Bnm = np.zeros((N_pad, GPC), NPBF16)
            Bnm[np.arange(n1 - n0), bl] = 1.0
            im[f'Bnm{d}'] = np.ascontiguousarray(
                Bnm.reshape(NCH, 128, GPC).transpose(1, 0, 2))
            im[f'Bt{d}'] = np.ascontiguousarray(Bnm.T)
            xt0 = np.zeros((64, N_pad), NPBF16)
            xt0[:, :n1 - n0] = _bf(x[d][n0:n1].T)
            im[f'xT{d}'] = xt0
            ut0 = np.ascontiguousarray(u_pad[d][c * GPC:(c + 1) * GPC].T)
            im[f'uT{d}'] = ut0
    return meta, in_maps


def _build(meta):
    N_pad, NCH, G = meta['N_pad'], meta['NCH'], meta['G']
    NE_CH, NE_CH_PAD, E_eff, NT = (meta['NE_CH'], meta['NE_CH_PAD'],
                                   meta['E_eff'], meta['NT'])
    node_tiles = []
    o = 0
    while o < N_pad:
        w = min(512, N_pad - o)
        node_tiles.append((o, w))
        o += w

    nc = bacc.Bacc("TRN2", target_bir_lowering=False, debug=False,
                   num_devices=NCORES, num_swdge_queues=4)
    DT = nc.dram_tensor
    ins = {}
    for nm, shp, dt in [
        ('we1a', [64, 128], BF16), ('we1b', [64, 128], BF16),
        ('we1ue', [128, 128], BF16), ('we1u1p', [64, 128], BF16),
        ('we1e', [64, 128], BF16), ('w1ua', [32, 128], BF16),
        ('w1ub', [32, 128], BF16), ('iotaG', [128, 4], F32),
        ('we2', [128, 128], BF16), ('we3', [128, 64], BF16),
        ('wn1x', [64, 128], BF16), ('wn1a', [64, 128], BF16),
        ('wn1ua', [32, 128], F32), ('wn1ub', [32, 128], F32),
        ('wn2', [128, 128], BF16), ('wn3', [128, 64], BF16),
        ('wa1', [64, 128], BF16), ('wa2', [128, 128], BF16), ('wa3', [128, 1], BF16),
        ('wg1p', [64, 128], F32), ('wg1a', [32, 128], F32), ('wg1b', [32, 128], F32),
        ('wg2', [128, 128], F32), ('wg3', [128, 32], F32),
        ('wo1a', [32, 128], F32), ('wo1b', [32, 128], F32),
        ('wo2', [128, 128], F32), ('wo3', [128, 2], F32),
        ('iota', [128, WIN], FP16), ('ident', [128, 128], F32),
        ('identb', [128, 128], BF16),
        ('wg1pb', [64, 128], BF16), ('wg1ab', [32, 128], BF16),
        ('wg1bb', [32, 128], BF16), ('wg2b', [128, 128], BF16),
        ('wg3b', [128, 32], BF16),
        ('xtbl0', [NCORES * N_pad, 128], BF16), ('xtbl1', [NCORES * N_pad, 128], BF16),
        ('utbl0', [GTOT, 128], BF16),
        ('uF0', [F_U, GTOT], BF16), ('uF1', [F_U, GTOT], BF16),
    ]:
        ins[nm] = DT(nm, shp, dt, kind="ExternalInput")
    for d in range(2):
        for nm, shp, dt in [
            (f'xsrcT{d}', [64, E_eff], BF16),
            (f'xdstT{d}', [64, E_eff], BF16),
            (f'ueT{d}', [128, E_eff], BF16),
            (f'srcw{d}', [128, E_eff // 16], I16),
            (f'dstw{d}', [128, E_eff // 16], I16),
            (f'gsrcw{d}', [128, E_eff // 16], I16),
            (f'gsrcb{d}', [128, E_eff], FP16),
            (f'dloc{d}', [128, NE_CH_PAD], FP16),
            (f'rcp{d}', [128, NE_CH_PAD], F32),
            (f'Bnm{d}', [128, NCH, GPC], BF16),
            (f'Bt{d}', [64, N_pad], BF16),
            (f'xT{d}', [64, N_pad], BF16),
            (f'uT{d}', [32, GPC], F32),
        ]:
            ins[nm] = DT(nm, shp, dt, kind="ExternalInput")
    out_d = DT("out", [N_STEPS, F_OUT, GPC], F32, kind="ExternalOutput")

    with tile.TileContext(nc) as tc:
        with (
            tc.tile_pool(name="const", bufs=1) as cpool,
            tc.tile_pool(name="pers", bufs=1) as pers,
            tc.tile_pool(name="gath", bufs=10) as gpool,
            tc.tile_pool(name="mlp", bufs=4) as mpool,
            tc.tile_pool(name="small", bufs=4) as spool,
            tc.tile_pool(name="ps_mlp", bufs=2, space="PSUM") as ps_mlp,
            tc.tile_pool(name="ps_l3", bufs=2, space="PSUM") as ps_l3,
            tc.tile_pool(name="ps_agg", bufs=2, space="PSUM") as ps_agg,
            tc.tile_pool(name="ps_pool", bufs=1, space="PSUM") as ps_pool,
            tc.tile_pool(name="dram", bufs=1, space="DRAM") as dpool,
        ):
            nc.gpsimd.load_library(library_config.mlp)

            def load_const(name):
                ap = ins[name]
                t = cpool.tile(list(ap.shape), ap.dtype, tag=name, name='c_' + name)
                nc.sync.dma_start(t[:], ap[:])
                return t

            W = {nm: load_const(nm) for nm in
                 ['we1a', 'we1b', 'we1ue', 'we1u1p', 'we2', 'we3', 'wn1x', 'wn1a',
                  'wn1ua', 'wn1ub', 'wn2', 'wn3',
                  'wa1', 'wa2', 'wa3', 'wg1p', 'wg1a', 'wg1b', 'wg2', 'wg3',
                  'wo1a', 'wo1b', 'wo2', 'wo3', 'iota', 'ident', 'identb',
                  'wg1pb', 'wg1ab', 'wg1bb', 'wg2b', 'wg3b',
                  'we1e', 'w1ua', 'w1ub', 'iotaG']}
            IDX = {}
            for d in range(2):
                for nm in [f'srcw{d}', f'dstw{d}', f'dloc{d}', f'rcp{d}',
                           f'Bnm{d}', f'Bt{d}']:
                    IDX[nm] = load_const(nm)

            xT = [[pers.tile([64, N_pad], BF16, tag=f'xT{d}_{s}',
                             name=f'xT{d}_{s}') for s in range(3)]
                  for d in range(2)]
            xnm = [pers.tile([128, NCH, 64], BF16, tag=f'xnm{d}', name=f'xnm{d}')
                   for d in range(2)]
            uT = [[pers.tile([32, GPC], F32, tag=f'uT{d}_{s}', name=f'uT{d}_{s}')
                   for s in range(3)]
                  for d in range(2)]
            uF = [[pers.tile([F_U, GTOT], BF16, tag=f'uF{d}_{k}', name=f'uF{d}_{k}')
                   for k in range(3)]
                  for d in range(2)]
            for d in range(2):
                nc.sync.dma_start(xT[d][0][:], ins[f'xT{d}'][:])
                nc.sync.dma_start(uT[d][0][:], ins[f'uT{d}'][:])
                nc.sync.dma_start(uF[d][0][:], ins[f'uF{d}'][:])

            eT_next = [dpool.tile([64, E_eff], BF16, tag=f'eTn{d}', name=f'eTn{d}')
                       for d in range(2)]
            xtbl_next = [dpool.tile([NCORES * N_pad, 128], BF16, tag=f'xtn{d}',
                                    name=f'xtn{d}')
                         for d in range(2)]
            utbl = {(0, 0): ins['utbl0']}
            for key in [(1, 0), (0, 1), (1, 1)]:
                utbl[key] = dpool.tile([GTOT, 128], BF16, tag=f'utbl{key[0]}{key[1]}',
                                       name=f'utbl{key[0]}{key[1]}')
            utbl_u1p = dpool.tile([GTOT, 128], BF16, tag='utblu1p', name='utblu1p')
            TuG = {k: pers.tile([128, 4, 128], BF16, tag=f'TuG{k[0]}{k[1]}',
                                name=f'TuG{k[0]}{k[1]}')
                   for k in [(1, 0), (0, 1), (1, 1)]}
            # boundary AllGathers: small pooled AG (urgent) + big x AG (lazy, s=0)
            agp = {}
            for key in [(0, 0), (1, 0), (0, 1)]:
                agp[key] = (dpool.tile([GPC, 64], BF16, name=f'agi{key[0]}{key[1]}',
                                       tag=f'agi{key[0]}{key[1]}'),
                            dpool.tile([GTOT, 64], BF16,
                                       name=f'ago{key[0]}{key[1]}',
                                       tag=f'ago{key[0]}{key[1]}'))
            agx = {d: (dpool.tile([N_pad, 64], BF16, name=f'agxi{d}', tag=f'agxi{d}'),
                       dpool.tile([NCORES * N_pad, 64], BF16, name=f'agxo{d}',
                                  tag=f'agxo{d}'))
                   for d in range(2)}

            RG = [list(range(NCORES))]

            def build_Tu(dest, parts):
                psT = ps_mlp.tile([128, 512], F32, tag='pmlp', space="PSUM",
                                  name='psT')
                for i, (wap, uap) in enumerate(parts):
                    nc.tensor.matmul(out=psT[:], lhsT=wap, rhs=uap,
                                     start=(i == 0), stop=(i == len(parts) - 1))
                TuT = spool.tile([128, 512], BF16, tag='TuT', bufs=2, name='TuT')
                nc.vector.tensor_copy(out=TuT[:], in_=psT[:])
                for a in range(4):
                    psxT = ps_pool.tile([128, GPC * 4], BF16, tag='ppool',
                                        space="PSUM", name='psxT')
                    nc.tensor.matmul(out=psxT[0:128, 0:128],
                                     lhsT=TuT[:, 128 * a:128 * a + 128],
                                     rhs=W['identb'][:], is_transpose=True,
                                     start=True, stop=True)
                    nc.vector.tensor_copy(out=dest[:, a, :],
                                          in_=psxT[0:128, 0:128])

            def repack_x(gout, table):
                nblk = NCORES * N_pad // 128
                half = nblk // 2
                rows = half * 128
                for hh in range(2):
                    bb = pers.tile([128, half, 64], BF16, tag='xbnc', name='xbnc')
                    nc.scalar.dma_start(
                        bb[:], gout[hh * rows:(hh + 1) * rows, :].rearrange(
                            "(c p) f -> p c f", p=128))
                    nc.scalar.dma_start(
                        table[hh * rows:(hh + 1) * rows, 0:64].rearrange(
                            "(c p) f -> p c f", p=128), bb[:])

            def transpose_32x512_to_gm(src, psdst_tag, identb):
                # src [32, 512] bf16 -> returns sbuf [128, 4, 32] bf16 graph-major
                outt = spool.tile([128, 4, 32], BF16, tag='ugm2', bufs=2, name='ugm2')
                for a in range(4):
                    pstx = ps_pool.tile([128, GPC * 4], BF16, tag='ppool', space="PSUM",
                                        name='pstx')
                    nc.tensor.matmul(out=pstx[0:128, 0:32],
                                     lhsT=src[:, 128 * a:128 * a + 128],
                                     rhs=identb[0:32, 0:32], is_transpose=True,
                                     start=True, stop=True)
                    nc.vector.tensor_copy(out=outt[:, a, :], in_=pstx[0:128, 0:32])
                return outt

            def build_utable(table, u_own, u_other, identb):
                for col0, src in ((0, u_own), (32, u_other)):
                    gm = transpose_32x512_to_gm(src, 'pstx', identb)
                    nc.sync.dma_start(
                        table[:, col0:col0 + 32].rearrange("(c p) f -> p c f", p=128),
                        gm[:])

            def gnn_step(d, s):
                xtbl_cur = ins[f'xtbl{d}'] if s == 0 else xtbl_next[d]
                eT_cur = eT_next[d]
                utbl_cur = utbl[(d, s)]
                srcw, dstw = IDX[f'srcw{d}'], IDX[f'dstw{d}']
                dlocs, rcps = IDX[f'dloc{d}'], IDX[f'rcp{d}']

                drains = pers.tile([64, NCH, WIN], BF16, tag='drains')
                aggT = pers.tile([64, N_pad], BF16, tag='aggT')
                psa_ref = [None]
                qrr = [0]

                def u_term_mms(ps1, tky, ecol):
                    gsb = gpool.tile([128, 512], FP16, tag='gsb', bufs=4, name='gsb')
                    nc.scalar.dma_start(gsb[:], ins[f'gsrcb{d}'][:, ecol])
                    for kt in range(4):
                        ohg = spool.tile([128, 512], BF16, tag='ohg', bufs=6,
                                         name='ohg')
                        nc.vector.tensor_scalar(out=ohg[:], in0=gsb[:],
                                                scalar1=W['iotaG'][:, kt:kt + 1],
                                                scalar2=None, op0=ALU.is_equal)
                        nc.tensor.matmul(out=ps1[:], lhsT=TuG[tky][:, kt, :],
                                         rhs=ohg[:], start=False, stop=(kt == 3))

                KPF = 10
                xg = {}

                def emit_xgather(t):
                    gsrc = gpool.tile([128, 1, 512], BF16, tag='gsrc', bufs=KPF + 2,
                                      name='gsrc')
                    gdst = gpool.tile([128, 1, 512], BF16, tag='gdst', bufs=KPF + 2,
                                      name='gdst')
                    ic = slice(32 * t, 32 * t + 32)
                    q = qrr[0]; qrr[0] = (q + 1) % 4
                    nc.gpsimd.dma_gather(gsrc[:], xtbl_cur[:], srcw[:, ic], 512, 512,
                                         128, transpose=True, queue_num=q)
                    q = qrr[0]; qrr[0] = (q + 1) % 4
                    nc.gpsimd.dma_gather(gdst[:], xtbl_cur[:], dstw[:, ic], 512, 512,
                                         128, transpose=True, queue_num=q)
                    xg[t] = (gsrc, gdst)

                if s == 1:
                    for t in range(min(KPF, NT)):
                        emit_xgather(t)
                for t in range(NT):
                    es = slice(0, 512)
                    ecol = slice(512 * t, 512 * t + 512)
                    ic = slice(32 * t, 32 * t + 32)
                    ps1 = ps_mlp.tile([128, 512], F32, tag='pmlp', space="PSUM")
                    if s == 0:
                        xs = gpool.tile([64, 512], BF16, tag='xs', bufs=4, name='xs')
                        xd = gpool.tile([64, 512], BF16, tag='xd', bufs=4, name='xd')
                        gue = gpool.tile([128, 512], BF16, tag='gue', bufs=4,
                                         name='gue')
                        nc.scalar.dma_start(xs[:], ins[f'xsrcT{d}'][:, ecol])
                        nc.scalar.dma_start(xd[:], ins[f'xdstT{d}'][:, ecol])
                        nc.sync.dma_start(gue[:], ins[f'ueT{d}'][:, ecol])
                        nc.tensor.matmul(out=ps1[:], lhsT=W['we1a'][:], rhs=xs[:],
                                         start=True, stop=False)
                        nc.tensor.matmul(out=ps1[:], lhsT=W['we1b'][:], rhs=xd[:],
                                         start=False, stop=False)
                        nc.tensor.matmul(out=ps1[:], lhsT=W['we1ue'][:], rhs=gue[:],
                                         start=False, stop=(d == 0))
                        if d == 1:
                            u_term_mms(ps1, (1, 0), ecol)
                    else:
                        gsrc, gdst = xg.pop(t)
                        if t + KPF < NT:
                            emit_xgather(t + KPF)
                        et = gpool.tile([64, 512], BF16, tag='et', bufs=4, name='et')
                        nc.sync.dma_start(et[:], eT_cur[:, ecol])
                        nc.tensor.matmul(out=ps1[:], lhsT=W['we1a'][:],
                                         rhs=gsrc[0:64, 0, es], start=True, stop=False)
                        nc.tensor.matmul(out=ps1[:], lhsT=W['we1b'][:],
                                         rhs=gdst[0:64, 0, es], start=False, stop=False)
                        nc.tensor.matmul(out=ps1[:], lhsT=W['we1e'][:],
                                         rhs=et[:], start=False, stop=False)
                        u_term_mms(ps1, (d, 1), ecol)
                    if True:
                      if True:
                        pass
                        h1 = mpool.tile([128, 512], BF16, tag='h1')
                        nc.scalar.activation(out=h1[:], in_=ps1[:], func=ACT.Relu)
                        ps2 = ps_mlp.tile([128, 512], F32, tag='pmlp', space="PSUM")
                        nc.tensor.matmul(out=ps2[:], lhsT=W['we2'][:], rhs=h1[:],
                                         start=True, stop=True)
                        h2 = mpool.tile([128, 512], BF16, tag='h2')
                        nc.scalar.activation(out=h2[:], in_=ps2[:], func=ACT.Relu)
                        if s == 0:
                            ps3 = ps_l3.tile([128, 512], F32, tag='pl3', space="PSUM")
                            nc.tensor.matmul(out=ps3[0:64, :], lhsT=W['we3'][:],
                                             rhs=h2[:], start=True, stop=True)
                            en = mpool.tile([64, 512], BF16, tag='en', bufs=2)
                            nc.vector.tensor_copy(out=en[:], in_=ps3[0:64, :])
                            nc.sync.dma_start(eT_next[d][:, 512 * t:512 * t + 512],
                                              en[:])
                        psd = ps_l3.tile([128, 512], F32, tag='pl3', space="PSUM")
                        for j in range(4):
                            if 4 * t + j >= NE_CH:
                                continue
                            nc.tensor.matmul(out=psd[:, 64 * j:64 * j + 64],
                                             lhsT=h2[:, 128 * j:128 * j + 128],
                                             rhs=W['we3'][:], start=True, stop=True)
                        eema = spool.tile([128, 4, 64], BF16, tag='eema', bufs=4,
                                          name='eema')
                        nc.vector.tensor_tensor(
                            out=eema[:],
                            in0=psd[:, 0:256].rearrange("p (c f) -> p c f", f=64),
                            in1=rcps[:, 4 * t:4 * t + 4, None].to_broadcast(
                                [128, 4, 64]),
                            op=ALU.mult)
                        for j in range(4):
                            k = 4 * t + j
                            if k >= NE_CH:
                                continue
                            g = k // G
                            w_in = k % G
                            oh = spool.tile([128, WIN], BF16, tag='oh', bufs=8)
                            nc.vector.tensor_tensor(
                                out=oh[:], in0=W['iota'][:],
                                in1=dlocs[:, k:k + 1].to_broadcast([128, WIN]),
                                op=ALU.is_equal)
                            if w_in == 0:
                                psa_ref[0] = ps_agg.tile([64, WIN], F32, tag='pagg',
                                                         space="PSUM", name='psa')
                            psa = psa_ref[0]
                            nc.tensor.matmul(out=psa[:], lhsT=eema[:, j, :], rhs=oh[:],
                                             start=(w_in == 0), stop=(w_in == G - 1))
                            if w_in == G - 1:
                                nc.vector.tensor_copy(out=drains[:, g, :], in_=psa[:])
                for cch in range(NCH):
                    if cch == 0:
                        nc.vector.tensor_copy(out=aggT[:, 0:128],
                                              in_=drains[:, 0, 0:128])
                    else:
                        nc.vector.tensor_add(out=aggT[:, 128 * cch:128 * cch + 128],
                                             in0=drains[:, cch, 0:128],
                                             in1=drains[:, cch - 1, 128:256])

                # UW[g, H] = u1c_gm[g] @ Wn1u, built as two K=32 matmuls
                psuw = ps_pool.tile([128, GPC * 2], F32, tag='ppool', space="PSUM")
                nc.tensor.matmul(out=psuw[0:GPC, 0:128], lhsT=uT[d][s][:],
                                 rhs=W['wn1ua'][:], start=True, stop=False)
                nc.tensor.matmul(out=psuw[0:GPC, 0:128], lhsT=uT[1 - d][s + d][:],
                                 rhs=W['wn1ub'][:], start=False, stop=True)
                UWb = spool.tile([GPC, 128], BF16, tag='uwb')
                nc.vector.tensor_copy(out=UWb[:], in_=psuw[0:GPC, 0:128])

                w_nm = pers.tile([128, NCH], F32, tag='w_nm')
                psP = ps_pool.tile([65, GPC], F32, tag='pP', space="PSUM")
                for (off, wdt) in node_tiles:
                    sl = slice(off, off + wdt)
                    psn = ps_mlp.tile([128, 512], F32, tag='pmlp', space="PSUM")
                    nc.tensor.matmul(out=psn[:, :wdt], lhsT=W['wn1x'][:],
                                     rhs=xT[d][s][:, sl], start=True, stop=False)
                    nc.tensor.matmul(out=psn[:, :wdt], lhsT=W['wn1a'][:],
                                     rhs=aggT[:, sl], start=False, stop=False)
                    nc.tensor.matmul(out=psn[:, :wdt], lhsT=UWb[:],
                                     rhs=IDX[f'Bt{d}'][:, sl], start=False, stop=True)
                    nh1 = mpool.tile([128, 512], BF16, tag='h1')
                    nc.scalar.activation(out=nh1[:, :wdt], in_=psn[:, :wdt],
                                         func=ACT.Relu)
                    psn2 = ps_mlp.tile([128, 512], F32, tag='pmlp', space="PSUM")
                    nc.tensor.matmul(out=psn2[:, :wdt], lhsT=W['wn2'][:],
                                     rhs=nh1[:, :wdt], start=True, stop=True)
                    nh2 = mpool.tile([128, 512], BF16, tag='h2')
                    nc.scalar.activation(out=nh2[:, :wdt], in_=psn2[:, :wdt],
                                         func=ACT.Relu)
                    psx = ps_l3.tile([128, 512], F32, tag='pl3', space="PSUM")
                    nc.tensor.matmul(out=psx[0:64, :wdt], lhsT=W['wn3'][:],
                                     rhs=nh2[:, :wdt], start=True, stop=True)
                    nc.vector.tensor_copy(out=xT[d][s + 1][:, sl], in_=psx[0:64, :wdt])
                    nch_here = wdt // 128
                    for j in range(nch_here):
                        cch = off // 128 + j
                        psd = ps_l3.tile([128, 512], F32, tag='pl3', space="PSUM")
                        nc.tensor.matmul(out=psd[:, 0:64],
                                         lhsT=nh2[:, 128 * j:128 * j + 128],
                                         rhs=W['wn3'][:], start=True, stop=True)
                        nc.vector.tensor_copy(out=xnm[d][:, cch, :], in_=psd[:, 0:64])
                    psa1 = ps_mlp.tile([128, 512], F32, tag='pmlp', space="PSUM")
                    nc.tensor.matmul(out=psa1[:, :wdt], lhsT=W['wa1'][:],
                                     rhs=xT[d][s + 1][:, sl], start=True, stop=True)
                    ah1 = mpool.tile([128, 512], BF16, tag='h1')
                    nc.scalar.activation(out=ah1[:, :wdt], in_=psa1[:, :wdt],
                                         func=ACT.Relu)
                    psa2 = ps_mlp.tile([128, 512], F32, tag='pmlp', space="PSUM")
                    nc.tensor.matmul(out=psa2[:, :wdt], lhsT=W['wa2'][:],
                                     rhs=ah1[:, :wdt], start=True, stop=True)
                    ah2 = mpool.tile([128, 512], BF16, tag='h2')
                    nc.scalar.activation(out=ah2[:, :wdt], in_=psa2[:, :wdt],
                                         func=ACT.Relu)
                    for j in range(nch_here):
                        cch = off // 128 + j
                        pss = ps_l3.tile([128, 512], F32, tag='pl3', space="PSUM")
                        nc.tensor.matmul(out=pss[:, 0:1],
                                         lhsT=ah2[:, 128 * j:128 * j + 128],
                                         rhs=W['wa3'][:], start=True, stop=True)
                        nc.scalar.activation(out=w_nm[:, cch:cch + 1], in_=pss[:, 0:1],
                                             func=ACT.Exp)
                        pw = spool.tile([128, 65], BF16, tag='pw')
                        nc.vector.tensor_scalar(out=pw[:, 0:64],
                                                in0=xnm[d][:, cch, :],
                                                scalar1=w_nm[:, cch:cch + 1],
                                                scalar2=None, op0=ALU.mult)
                        nc.vector.tensor_copy(out=pw[:, 64:65],
                                              in_=w_nm[:, cch:cch + 1])
                        nc.tensor.matmul(out=psP[:], lhsT=pw[:],
                                         rhs=IDX[f'Bnm{d}'][:, cch, :],
                                         start=(cch == 0), stop=(cch == NCH - 1))

                pooln = spool.tile([65, GPC], F32, tag='pooln')
                nc.vector.tensor_copy(out=pooln[:], in_=psP[:])
                pst = ps_pool.tile([128, GPC * 2], F32, tag='ppool', space="PSUM")
                nc.tensor.matmul(out=pst[0:GPC, 0:65], lhsT=pooln[:],
                                 rhs=W['ident'][0:65, 0:65],
                                 is_transpose=True, start=True, stop=True)
                pgm = spool.tile([GPC, 65], F32, tag='pgm')
                nc.vector.tensor_copy(out=pgm[:], in_=pst[0:GPC, 0:65])
                dn = spool.tile([GPC, 1], F32, tag='dn')
                nc.vector.tensor_scalar(out=dn[:], in0=pgm[:, 64:65], scalar1=1e-16,
                                        scalar2=None, op0=ALU.max)
                rdn = spool.tile([GPC, 1], F32, tag='rdn')
                nc.vector.reciprocal(out=rdn[:], in_=dn[:])
                pgs = spool.tile([GPC, 64], F32, tag='pgs')
                nc.vector.tensor_scalar(out=pgs[:], in0=pgm[:, 0:64], scalar1=rdn[:],
                                        scalar2=None, op0=ALU.mult)
                psb = ps_pool.tile([128, GPC * 2], F32, tag='ppool', space="PSUM")
                nc.tensor.matmul(out=psb[0:64, 0:GPC], lhsT=pgs[:],
                                 rhs=W['ident'][0:GPC, 0:GPC],
                                 is_transpose=True, start=True, stop=True)
                pooledT = spool.tile([64, GPC], F32, tag='pooledT')
                nc.vector.tensor_copy(out=pooledT[:], in_=psb[0:64, 0:GPC])
                psg = ps_pool.tile([128, GPC * 2], F32, tag='ppool', space="PSUM")
                nc.tensor.matmul(out=psg[:, 0:GPC], lhsT=W['wg1p'][:], rhs=pooledT[:],
                                 start=True, stop=False)
                nc.tensor.matmul(out=psg[:, 0:GPC], lhsT=W['wg1a'][:], rhs=uT[d][s][:],
                                 start=False, stop=False)
                nc.tensor.matmul(out=psg[:, 0:GPC], lhsT=W['wg1b'][:],
                                 rhs=uT[1 - d][s + d][:], start=False, stop=True)
                gh1 = spool.tile([128, GPC], F32, tag='gh1')
                nc.scalar.activation(out=gh1[:], in_=psg[:, 0:GPC], func=ACT.Relu)
                psg2 = ps_pool.tile([128, GPC * 2], F32, tag='ppool', space="PSUM")
                nc.tensor.matmul(out=psg2[:, 0:GPC], lhsT=W['wg2'][:], rhs=gh1[:],
                                 start=True, stop=True)
                gh2 = spool.tile([128, GPC], F32, tag='gh2')
                nc.scalar.activation(out=gh2[:], in_=psg2[:, 0:GPC], func=ACT.Relu)
                psg3 = ps_pool.tile([128, GPC * 2], F32, tag='ppool', space="PSUM")
                nc.tensor.matmul(out=psg3[0:32, 0:GPC], lhsT=W['wg3'][:], rhs=gh2[:],
                                 start=True, stop=True)
                nc.vector.tensor_copy(out=uT[d][s + 1][:], in_=psg3[0:32, 0:GPC])

                # ---- boundary: small pooled AG first, big x AG after ----
                if (d, s) != (1, 1):
                    pgsb = spool.tile([GPC, 64], BF16, tag='pgsb')
                    nc.vector.tensor_copy(out=pgsb[:], in_=pgs[:])
                    gin_, gout_ = agp[(d, s)]
                    nc.sync.dma_start(gin_[:], pgsb[:])
                    nc.gpsimd.collective_compute(
                        "AllGather", ALU.bypass, replica_groups=RG,
                        ins=[gin_[:].opt()], outs=[gout_[:].opt()])
                    if s == 0:
                        xgi, xgo = agx[d]
                        nc.scalar.dma_start(
                            xgi[:].rearrange("(c p) f -> p c f", p=128), xnm[d][:])
                        nc.gpsimd.collective_compute(
                            "AllGather", ALU.bypass, replica_groups=RG,
                            ins=[xgi[:].opt()], outs=[xgo[:].opt()])
                        repack_x(xgo, xtbl_next[d])
                    # pooled for all graphs -> sbuf graph-major [64, 8, 64]
                    pAll = spool.tile([GPC, NCORES, 64], BF16, tag='pAll', bufs=2)
                    for a in range(NCORES):
                        nc.sync.dma_start(pAll[:, a, :],
                                          gout_[a * GPC:(a + 1) * GPC, :])
                    # pooledT_full [64 f, 512 g]
                    pTf = spool.tile([64, GTOT], BF16, tag='pTf', bufs=2)
                    for a in range(NCORES):
                        pstp = ps_pool.tile([128, GPC * 4], BF16, tag='ppool',
                                            space="PSUM", name='pstp')
                        nc.tensor.matmul(out=pstp[0:64, 0:GPC], lhsT=pAll[:, a, :],
                                         rhs=W['identb'][0:GPC, 0:GPC],
                                         is_transpose=True, start=True, stop=True)
                        nc.vector.tensor_copy(out=pTf[:, GPC * a:GPC * (a + 1)],
                                              in_=pstp[0:64, 0:GPC])
                    # replicated glob MLP (bf16): uF[d][s+1] for ALL graphs
                    uFA = uF[d][s]
                    uFB = uF[1 - d][s + d]
                    psG = ps_mlp.tile([128, 512], F32, tag='pmlp', space="PSUM",
                                      name='psG')
                    nc.tensor.matmul(out=psG[:], lhsT=W['wg1pb'][:], rhs=pTf[:],
                                     start=True, stop=False)
                    nc.tensor.matmul(out=psG[:], lhsT=W['wg1ab'][:], rhs=uFA[:],
                                     start=False, stop=False)
                    nc.tensor.matmul(out=psG[:], lhsT=W['wg1bb'][:], rhs=uFB[:],
                                     start=False, stop=True)
                    gH1 = mpool.tile([128, 512], BF16, tag='h1', name='gH1')
                    nc.scalar.activation(out=gH1[:], in_=psG[:], func=ACT.Relu)
                    psG2 = ps_mlp.tile([128, 512], F32, tag='pmlp', space="PSUM",
                                       name='psG2')
                    nc.tensor.matmul(out=psG2[:], lhsT=W['wg2b'][:], rhs=gH1[:],
                                     start=True, stop=True)
                    gH2 = mpool.tile([128, 512], BF16, tag='h2', name='gH2')
                    nc.scalar.activation(out=gH2[:], in_=psG2[:], func=ACT.Relu)
                    psG3 = ps_l3.tile([128, 512], F32, tag='pl3', space="PSUM",
                                      name='psG3')
                    nc.tensor.matmul(out=psG3[0:F_U, :], lhsT=W['wg3b'][:], rhs=gH2[:],
                                     start=True, stop=True)
                    nc.vector.tensor_copy(out=uF[d][s + 1][:], in_=psG3[0:F_U, :])
                    # build the u-table for the NEXT gnn_step
                    nxt = {(0, 0): (1, 0), (1, 0): (0, 1), (0, 1): (1, 1)}[(d, s)]
                    if (d, s) == (0, 0):
                        build_Tu(TuG[(1, 0)], [(W['w1ub'][:], uF[0][1][:])])
                    elif (d, s) == (1, 0):
                        build_Tu(TuG[(0, 1)], [(W['w1ua'][:], uF[0][1][:]),
                                               (W['w1ub'][:], uF[1][1][:])])
                    else:
                        build_Tu(TuG[(1, 1)], [(W['w1ua'][:], uF[1][1][:]),
                                               (W['w1ub'][:], uF[0][2][:])])

            for s in range(N_STEPS):
                for d in range(2):
                    gnn_step(d, s)
                pso = ps_pool.tile([128, GPC * 2], F32, tag='ppool', space="PSUM")
                nc.tensor.matmul(out=pso[:, 0:GPC], lhsT=W['wo1a'][:],
                                 rhs=uT[0][s + 1][:], start=True, stop=False)
                nc.tensor.matmul(out=pso[:, 0:GPC], lhsT=W['wo1b'][:],
                                 rhs=uT[1][s + 1][:], start=False, stop=True)
                oh1 = spool.tile([128, GPC], F32, tag='oh1')
                nc.scalar.activation(out=oh1[:], in_=pso[:, 0:GPC], func=ACT.Relu)
                pso2 = ps_pool.tile([128, GPC * 2], F32, tag='ppool', space="PSUM")
                nc.tensor.matmul(out=pso2[:, 0:GPC], lhsT=W['wo2'][:], rhs=oh1[:],
                                 start=True, stop=True)
                oh2 = spool.tile([128, GPC], F32, tag='oh2')
                nc.scalar.activation(out=oh2[:], in_=pso2[:, 0:GPC], func=ACT.Relu)
                pso3 = ps_pool.tile([128, GPC * 2], F32, tag='ppool', space="PSUM")
                nc.tensor.matmul(out=pso3[0:F_OUT, 0:GPC], lhsT=W['wo3'][:], rhs=oh2[:],
                                 start=True, stop=True)
                ot = spool.tile([F_OUT, GPC], F32, tag='ot')
                nc.vector.tensor_copy(out=ot[:], in_=pso3[0:F_OUT, 0:GPC])
                nc.sync.dma_start(out_d[s], ot[:])
    nc.compile()
    return nc


_CACHE = {}


def _get_nc(meta):
    key = tuple(sorted(meta.items()))
    if key not in _CACHE:
        _CACHE[key] = _build(meta)
    return _CACHE[key]


def kernel(**inputs):
    meta, in_maps = _prep(inputs)
    nc = _get_nc(meta)
    res = run_bass_kernel_spmd(nc, in_maps, core_ids=list(range(NCORES)))
    out = np.zeros((N_STEPS, B, F_OUT), np.float32)
    for c in range(NCORES):
        o = res.results[c]["out"]
        g0 = c * GPC
        g1 = min(B, g0 + GPC)
        if g1 > g0:
            out[:, g0:g1, :] = np.transpose(o, (0, 2, 1))[:, :g1 - g0, :]
    return out
